# revision 18
# baseline (speedup 1.0000x reference)
"""Trainium2 Bass kernel for batched cross-attention (B=8, Lq=1024, Lk=2048, D=1024).

Sharding: pure data-parallel over the batch dim — each of the 8 NeuronCores
computes full attention for one batch element. Weights are replicated.

Per core:
  Q = q_b @ Wq^T + bq          [1024, 1024]
  K = x_b @ Wk^T + bk          [2048, 1024]
  V = x_b @ Wv^T + bv          [2048, 1024]
  S = Q @ K^T / sqrt(D)        [1024, 2048]
  A = softmax(S, axis=-1)      [1024, 2048]  (output 2)
  O = A @ V                    [1024, 1024]  (output 1)

All matmuls run as float32r (full-rate fp32 on the PE at moving-dim>=256).
Operand transposes (contraction dim must sit on SBUF partitions) are done on
the TensorEngine via identity-matmul transposes; groups of 4 transposes share
one PSUM bank so each PSUM->SBUF copy moves [128, 4, 128] at once.

Phase A reads x once and computes BOTH K^T (kept in SBUF) and V. V is bounced
through DRAM: its write happens in PE-bound phase A, its read at the start of
DMA-slack phase B — this keeps phase A's DMA under the ~200GB/s ceiling.

The weighted matmul consumes the *unnormalized* exp(S) transpose; the softmax
1/rowsum is folded into the PSUM->SBUF copy of the weighted output as a
per-partition scale, so the PE never waits on the softmax reduction.
"""

import sys

import numpy as np

if "/opt/trn_rl_repo" not in sys.path:
    sys.path.insert(0, "/opt/trn_rl_repo")

P = 128
D = 1024  # model dim
LQ = 1024  # query length
LK = 2048  # key length
B = 8  # batch == number of cores
DC = D // P  # 8 chunks of the contraction (d) dim
EC = D // P  # 8 chunks of the output-feature (e) dim
KC = LK // P  # 16 chunks of the key dim
KB = 512  # k-block width for phase A streaming
NKB = LK // KB  # 4
NQB = LQ // P  # 8 query blocks
SCALE = 1.0 / 32.0  # 1/sqrt(D)

_CACHE = {}


def build_nc(use_bias=True):
    import concourse.mybir as mybir
    import concourse.tile as tile
    from concourse import bacc
    from concourse.masks import make_identity

    F32 = mybir.dt.float32
    F32R = mybir.dt.float32r
    Exp = mybir.ActivationFunctionType.Exp
    Ident = mybir.ActivationFunctionType.Identity
    Copy = mybir.ActivationFunctionType.Copy
    AX = mybir.AxisListType.X

    nc = bacc.Bacc(
        "TRN2",
        target_bir_lowering=False,
        debug=False,
        enable_asserts=False,
        num_devices=B,
    )

    x_d = nc.dram_tensor("x", [LK, D], F32, kind="ExternalInput").ap()
    q_d = nc.dram_tensor("q", [LQ, D], F32, kind="ExternalInput").ap()
    wq_d = nc.dram_tensor("Wq", [D, D], F32, kind="ExternalInput").ap()
    bq_d = nc.dram_tensor("bq", [D], F32, kind="ExternalInput").ap()
    wk_d = nc.dram_tensor("Wk", [D, D], F32, kind="ExternalInput").ap()
    bk_d = nc.dram_tensor("bk", [D], F32, kind="ExternalInput").ap()
    wv_d = nc.dram_tensor("Wv", [D, D], F32, kind="ExternalInput").ap()
    bv_d = nc.dram_tensor("bv", [D], F32, kind="ExternalInput").ap()
    out_d = nc.dram_tensor("weighted", [LQ, D], F32, kind="ExternalOutput").ap()
    attn_d = nc.dram_tensor("attention", [LQ, LK], F32, kind="ExternalOutput").ap()
    # V bounce, e-half-major so phase B can fetch each half contiguously:
    # V_dram[eh, p, ko, j] = V[ko*128 + p, eh*512 + j]
    v_dram = nc.dram_tensor("V_scratch", [2, P, KC, 512], F32R).ap()

    with tile.TileContext(nc) as tc:
        with (
            tc.tile_pool(name="const", bufs=1) as cpool,
            tc.tile_pool(name="persist", bufs=1) as persist,
            tc.tile_pool(name="psumT4", bufs=3, space="PSUM") as psumT4,
            tc.tile_pool(name="psumMM", bufs=4, space="PSUM") as psumMM,
        ):
            ident = cpool.tile([P, P], F32, tag="ident")
            make_identity(nc, ident[:])

            if use_bias:
                # per-partition bias layouts: b[(eo p)] -> [p, eo]
                bk_sb = cpool.tile([P, EC], F32, tag="bk")
                nc.sync.dma_start(bk_sb[:], bk_d.rearrange("(o p) -> p o", p=P))
                bq_sb = cpool.tile([P, EC], F32, tag="bq")
                nc.sync.dma_start(bq_sb[:], bq_d.rearrange("(o p) -> p o", p=P))

                # ones-column trick operands for the V bias (free-dim bias):
                # onescol[p, m] = (p == 0); bvpad[0, :] = bv, others zero.
                # psum += onescol.T @ bvpad broadcasts bv to all partitions.
                onescol = cpool.tile([P, P], F32R, tag="onescol")
                bvpad = cpool.tile([P, D], F32R, tag="bvpad")

            # K^T [e, k] and Wq^T stay resident from phase A through phase B.
            KT = persist.tile([P, EC, LK], F32R, tag="KT")  # KT[p, eo, k]
            WqT = persist.tile([P, DC, D], F32R, tag="WqT")  # WqT[p, do, e]

            def dma_chunk_split(dst, src_row0, src_ap):
                """DMA a [P, D] natural chunk in two halves so transposes of
                the first half can start while the second half streams."""
                nc.sync.dma_start(
                    dst[:, 0 : D // 2],
                    src_ap[src_row0 : src_row0 + P, 0 : D // 2],
                )
                nc.sync.dma_start(
                    dst[:, D // 2 : D],
                    src_ap[src_row0 : src_row0 + P, D // 2 : D],
                )

            def transpose_batch(src, src_off, dst4, dst_c0, nblk):
                """PE-transpose `nblk` (<=4) contiguous [P, P] blocks of `src`
                starting at free-offset `src_off`, into dst4[:, dst_c0+j, :]
                via one shared PSUM bank and a single batched copy."""
                pst = psumT4.tile([P, 4, P], F32, tag="pT4")
                for j in range(nblk):
                    nc.tensor.transpose(
                        pst[:, j, :],
                        src[:, src_off + j * P : src_off + (j + 1) * P],
                        ident[:],
                    )
                nc.any.tensor_copy(
                    dst4[:, dst_c0 : dst_c0 + nblk, :], pst[:, :nblk, :]
                )

            # ------------- Phase A: K^T (SBUF) + V (DRAM) from one x pass ----
            with (
                tc.tile_pool(name="pa_nat", bufs=2) as natp,
                tc.tile_pool(name="pa_wT", bufs=1) as wTp,
                tc.tile_pool(name="pa_xT", bufs=2) as xTp,
                tc.tile_pool(name="pa_vstg", bufs=2) as vstgp,
            ):
                # PE warmup: ~3.5us of dummy fp32 matmuls on the identity so
                # the HAM clock-gate reaches 8/8 while the first DMAs stream.
                wps = psumMM.tile([P, 512], F32, tag="pMM")
                for w in range(8):
                    nc.tensor.matmul(
                        wps[:, 0:P],
                        ident[:],
                        ident[:],
                        start=(w == 0),
                        stop=(w == 7),
                    )

                if use_bias:
                    # f32r tiles can't be memset/DMA'd directly: stage in f32
                    # and let ACT copies do the f32->f32r rounding.
                    stage = natp.tile([P, D], F32, tag="nat")
                    nc.gpsimd.memset(stage[:], 0.0)
                    nc.gpsimd.memset(stage[0:1, 0:P], 1.0)
                    nc.scalar.copy(onescol[:], stage[:, 0:P])
                    bvstage = natp.tile([P, D], F32, tag="nat")
                    nc.gpsimd.memset(bvstage[:], 0.0)
                    nc.sync.dma_start(
                        bvstage[0:1, :], bv_d.rearrange("(a d) -> a d", a=1)
                    )
                    nc.scalar.copy(bvpad[:], bvstage[:])

                WkT = wTp.tile([P, DC, D], F32R, tag="WkT")
                WvT = wTp.tile([P, DC, D], F32R, tag="WvT")
                WkT4 = WkT[:].rearrange("p dc (g b) -> p (dc g) b", b=P)
                WvT4 = WvT[:].rearrange("p dc (g b) -> p (dc g) b", b=P)

                def transpose_w(wdram, wT4):
                    for eo in range(EC):
                        wchunk = natp.tile([P, D], F32, tag="nat")
                        dma_chunk_split(wchunk, eo * P, wdram)
                        for dc in range(DC):
                            transpose_batch(wchunk, dc * P, wT4, dc * EC + eo, 1)

                def transpose_x_block(kb):
                    xT = xTp.tile([P, DC, KB], F32R, tag="xT")
                    xT4 = xT[:].rearrange("p dc (g b) -> p (dc g) b", b=P)
                    for k4 in range(KB // P):
                        xchunk = natp.tile([P, D], F32, tag="nat")
                        dma_chunk_split(xchunk, kb * KB + k4 * P, x_d)
                        for dc in range(DC):
                            transpose_batch(
                                xchunk, dc * P, xT4, dc * (KB // P) + k4, 1
                            )
                    return xT

                def kt_block(kb, xT):
                    # KT[:, ec, kb] += sum_dc WkT[:, dc, ec].T @ xT[:, dc, :]
                    for ec in range(EC):
                        ps = psumMM.tile([P, 512], F32, tag="pMM")
                        for dc in range(DC):
                            nc.tensor.matmul(
                                ps[:, :KB],
                                WkT[:, dc, ec * P : (ec + 1) * P],
                                xT[:, dc, :],
                                start=(dc == 0),
                                stop=(dc == DC - 1),
                            )
                        if use_bias:
                            # copy + per-partition bias bk[e]
                            nc.scalar.activation(
                                KT[:, ec, kb * KB : (kb + 1) * KB],
                                ps[:, :KB],
                                Ident,
                                bias=bk_sb[:, ec : ec + 1],
                            )
                        else:
                            nc.any.tensor_copy(
                                KT[:, ec, kb * KB : (kb + 1) * KB], ps[:, :KB]
                            )

                def v_block(kb, xT):
                    # V[kb*4+k4] = sum_dc xT[:,dc,k4].T @ WvT[:,dc,:] (+ bv),
                    # staged out to DRAM for phase B
                    for k4 in range(KB // P):
                        ko = kb * (KB // P) + k4
                        for eh in range(2):
                            ps = psumMM.tile([P, 512], F32, tag="pMM")
                            if use_bias:
                                nc.tensor.matmul(
                                    ps[:],
                                    onescol[:],
                                    bvpad[:, eh * 512 : (eh + 1) * 512],
                                    start=True,
                                    stop=False,
                                )
                            for dc in range(DC):
                                nc.tensor.matmul(
                                    ps[:],
                                    xT[:, dc, k4 * P : (k4 + 1) * P],
                                    WvT[:, dc, eh * 512 : (eh + 1) * 512],
                                    start=(dc == 0 and not use_bias),
                                    stop=(dc == DC - 1),
                                )
                            vstg = vstgp.tile([P, 512], F32R, tag="vstg")
                            nc.any.tensor_copy(vstg[:], ps[:])
                            nc.sync.dma_start(v_dram[eh, :, ko, :], vstg[:])

                # Ordering: Wk streams first so KT matmuls can start after only
                # 4MB of W traffic; Wv streams while KT(kb0) computes.
                transpose_w(wk_d, WkT4)
                xT0 = transpose_x_block(0)
                kt_block(0, xT0)
                transpose_w(wv_d, WvT4)
                v_block(0, xT0)
                for kb in range(1, NKB):
                    xT = transpose_x_block(kb)
                    kt_block(kb, xT)
                    v_block(kb, xT)

                # build Wq^T at the tail of phase A: its DMA rides the DMA
                # slack here, and phase B then starts with operands ready
                WqT4 = WqT[:].rearrange("p dc (g b) -> p (dc g) b", b=P)
                transpose_w(wq_d, WqT4)

            # ------------- Phase B: per-q-block attention --------------------
            with (
                tc.tile_pool(name="pb_v", bufs=1) as vp,
                tc.tile_pool(name="pb_nat", bufs=2) as qnatp,
                tc.tile_pool(name="pb_small", bufs=1) as smallp,
                tc.tile_pool(name="pb_exp", bufs=1) as expp,
                tc.tile_pool(name="pb_attnT", bufs=1) as attnTp,
                tc.tile_pool(name="pb_out", bufs=1) as outp,
            ):
                # fetch V from the bounce buffer, e-half 0 first (the first
                # weighted matmul needs only half 0)
                V = vp.tile([P, KC, D], F32R, tag="V")  # V[p, ko, e]
                for eh in range(2):
                    nc.sync.dma_start(
                        V[:, :, eh * 512 : (eh + 1) * 512], v_dram[eh]
                    )

                # keep the PE warm across the phase boundary while the V/q
                # DMAs land
                wps = psumMM.tile([P, 512], F32, tag="pMM")
                for w in range(8):
                    nc.tensor.matmul(
                        wps[:, 0:P],
                        ident[:],
                        ident[:],
                        start=(w == 0),
                        stop=(w == 7),
                    )

                for qb in range(NQB):
                    qs = qb * P
                    # load q block, transpose to qT[d-part, q]
                    qchunk = qnatp.tile([P, D], F32, tag="qnat")
                    dma_chunk_split(qchunk, qs, q_d)
                    qT = smallp.tile([P, DC, P], F32R, tag="qT")
                    for g in range(DC // 4):
                        transpose_batch(qchunk, g * 4 * P, qT, g * 4, 4)

                    # Q natural [q, e] = sum_dc qT[:, dc].T @ WqT[:, dc, :]
                    Qn = smallp.tile([P, D], F32, tag="Qn")
                    for eh in range(2):
                        ps = psumMM.tile([P, 512], F32, tag="pMM")
                        for dc in range(DC):
                            nc.tensor.matmul(
                                ps[:],
                                qT[:, dc, :],
                                WqT[:, dc, eh * 512 : (eh + 1) * 512],
                                start=(dc == 0),
                                stop=(dc == DC - 1),
                            )
                        nc.any.tensor_copy(Qn[:, eh * 512 : (eh + 1) * 512], ps[:])

                    # transpose Q -> QT[e-part, q], adding bq (per-partition)
                    QT = smallp.tile([P, EC, P], F32R, tag="QT")
                    if use_bias:
                        for g in range(EC // 4):
                            pst = psumT4.tile([P, 4, P], F32, tag="pT4")
                            for j in range(4):
                                ec = g * 4 + j
                                nc.tensor.transpose(
                                    pst[:, j, :],
                                    Qn[:, ec * P : (ec + 1) * P],
                                    ident[:],
                                )
                            for j in range(4):
                                ec = g * 4 + j
                                nc.scalar.activation(
                                    QT[:, ec, :],
                                    pst[:, j, :],
                                    Ident,
                                    bias=bq_sb[:, ec : ec + 1],
                                )
                    else:
                        for g in range(EC // 4):
                            transpose_batch(Qn, g * 4 * P, QT, g * 4, 4)

                    # scores (psum) -> exp + row-sum, in chunks of 512
                    exp_sb = expp.tile([P, LK], F32, tag="exp")
                    sums4 = smallp.tile([P, 4], F32, tag="sums4")
                    for kq in range(LK // 512):
                        ps = psumMM.tile([P, 512], F32, tag="pMM")
                        for ec in range(EC):
                            nc.tensor.matmul(
                                ps[:],
                                QT[:, ec, :],
                                KT[:, ec, kq * 512 : (kq + 1) * 512],
                                start=(ec == 0),
                                stop=(ec == EC - 1),
                            )
                        nc.scalar.activation(
                            exp_sb[:, kq * 512 : (kq + 1) * 512],
                            ps[:],
                            Exp,
                            scale=SCALE,
                            accum_out=sums4[:, kq : kq + 1],
                        )

                    # transpose UNNORMALIZED exp -> attnT[k-part, q]; the
                    # 1/rowsum is applied on the weighted output instead,
                    # so the PE never waits on the softmax reduction.
                    attnT = attnTp.tile([P, KC, P], F32R, tag="attnT")
                    for g in range(KC // 4):
                        transpose_batch(exp_sb, g * 4 * P, attnT, g * 4, 4)

                    sumk = smallp.tile([P, 1], F32, tag="sumk")
                    nc.vector.reduce_sum(sumk[:], sums4[:], axis=AX)
                    rsum = smallp.tile([P, 1], F32, tag="rsum")
                    nc.vector.reciprocal(rsum[:], sumk[:])
                    # normalize in place (after the transposes read it),
                    # write attention out
                    nc.vector.tensor_scalar_mul(exp_sb[:], exp_sb[:], rsum[:])
                    nc.sync.dma_start(attn_d[qs : qs + P, :], exp_sb[:])

                    # weighted [q, e] = (sum_kc attnT[:, kc].T @ V) * rsum
                    wout = outp.tile([P, D], F32, tag="wout")
                    for eh in range(2):
                        ps = psumMM.tile([P, 512], F32, tag="pMM")
                        for kc in range(KC):
                            nc.tensor.matmul(
                                ps[:],
                                attnT[:, kc, :],
                                V[:, kc, eh * 512 : (eh + 1) * 512],
                                start=(kc == 0),
                                stop=(kc == KC - 1),
                            )
                        nc.scalar.activation(
                            wout[:, eh * 512 : (eh + 1) * 512],
                            ps[:],
                            Copy,
                            scale=rsum[:],
                        )
                    nc.sync.dma_start(out_d[qs : qs + P, :], wout[:])

    nc.compile()
    return nc


def _get_nc(use_bias=True):
    key = ("nc", use_bias)
    if key not in _CACHE:
        _CACHE[key] = build_nc(use_bias=use_bias)
    return _CACHE[key]


def kernel(**inputs):
    from concourse.bass_utils import run_bass_kernel_spmd

    use_bias = any(
        np.any(np.asarray(inputs[k])) for k in ("bq", "bk", "bv")
    )
    nc = _get_nc(use_bias=use_bias)
    in_maps = []
    for b in range(B):
        in_maps.append(
            {
                "x": np.ascontiguousarray(inputs["x"][b], dtype=np.float32),
                "q": np.ascontiguousarray(inputs["q"][b], dtype=np.float32),
                "Wq": np.asarray(inputs["Wq"], dtype=np.float32),
                "bq": np.asarray(inputs["bq"], dtype=np.float32),
                "Wk": np.asarray(inputs["Wk"], dtype=np.float32),
                "bk": np.asarray(inputs["bk"], dtype=np.float32),
                "Wv": np.asarray(inputs["Wv"], dtype=np.float32),
                "bv": np.asarray(inputs["bv"], dtype=np.float32),
            }
        )
    res = run_bass_kernel_spmd(nc, in_maps, core_ids=list(range(B)))
    weighted = np.stack([res.results[b]["weighted"] for b in range(B)])
    attention = np.stack([res.results[b]["attention"] for b in range(B)])
    return weighted, attention


# revision 19
# speedup vs baseline: 1.0901x; 1.0901x over previous
"""Trainium2 Bass kernel for batched cross-attention (B=8, Lq=1024, Lk=2048, D=1024).

Sharding: pure data-parallel over the batch dim — each of the 8 NeuronCores
computes full attention for one batch element. Weights are replicated.

Per core:
  Q = q_b @ Wq^T + bq          [1024, 1024]
  K = x_b @ Wk^T + bk          [2048, 1024]
  V = x_b @ Wv^T + bv          [2048, 1024]
  S = Q @ K^T / sqrt(D)        [1024, 2048]
  A = softmax(S, axis=-1)      [1024, 2048]  (output 2)
  O = A @ V                    [1024, 1024]  (output 1)

All matmuls run as float32r (full-rate fp32 on the PE at moving-dim>=256).
Operand transposes (contraction dim must sit on SBUF partitions) are done on
the TensorEngine via identity-matmul transposes; groups of 4 transposes share
one PSUM bank so each PSUM->SBUF copy moves [128, 4, 128] at once.

Phase A reads x once and computes BOTH K^T (kept in SBUF) and V. V is bounced
through DRAM: its write happens in PE-bound phase A, its read at the start of
DMA-slack phase B — this keeps phase A's DMA under the ~200GB/s ceiling.

The weighted matmul consumes the *unnormalized* exp(S) transpose; the softmax
1/rowsum is folded into the PSUM->SBUF copy of the weighted output as a
per-partition scale, so the PE never waits on the softmax reduction.
"""

import sys

import numpy as np

if "/opt/trn_rl_repo" not in sys.path:
    sys.path.insert(0, "/opt/trn_rl_repo")

P = 128
D = 1024  # model dim
LQ = 1024  # query length
LK = 2048  # key length
B = 8  # batch == number of cores
DC = D // P  # 8 chunks of the contraction (d) dim
EC = D // P  # 8 chunks of the output-feature (e) dim
KC = LK // P  # 16 chunks of the key dim
KB = 512  # k-block width for phase A streaming
NKB = LK // KB  # 4
NQB = LQ // P  # 8 query blocks
SCALE = 1.0 / 32.0  # 1/sqrt(D)

_CACHE = {}


def build_nc(use_bias=True):
    import concourse.mybir as mybir
    import concourse.tile as tile
    from concourse import bacc
    from concourse.masks import make_identity

    F32 = mybir.dt.float32
    F32R = mybir.dt.float32r
    Exp = mybir.ActivationFunctionType.Exp
    Ident = mybir.ActivationFunctionType.Identity
    Copy = mybir.ActivationFunctionType.Copy
    AX = mybir.AxisListType.X

    nc = bacc.Bacc(
        "TRN2",
        target_bir_lowering=False,
        debug=False,
        enable_asserts=False,
        num_devices=B,
    )

    x_d = nc.dram_tensor("x", [LK, D], F32, kind="ExternalInput").ap()
    q_d = nc.dram_tensor("q", [LQ, D], F32, kind="ExternalInput").ap()
    wq_d = nc.dram_tensor("Wq", [D, D], F32, kind="ExternalInput").ap()
    bq_d = nc.dram_tensor("bq", [D], F32, kind="ExternalInput").ap()
    wk_d = nc.dram_tensor("Wk", [D, D], F32, kind="ExternalInput").ap()
    bk_d = nc.dram_tensor("bk", [D], F32, kind="ExternalInput").ap()
    wv_d = nc.dram_tensor("Wv", [D, D], F32, kind="ExternalInput").ap()
    bv_d = nc.dram_tensor("bv", [D], F32, kind="ExternalInput").ap()
    out_d = nc.dram_tensor("weighted", [LQ, D], F32, kind="ExternalOutput").ap()
    attn_d = nc.dram_tensor("attention", [LQ, LK], F32, kind="ExternalOutput").ap()
    # V bounce, e-half-major so phase B can fetch each half contiguously:
    # V_dram[eh, p, ko, j] = V[ko*128 + p, eh*512 + j]
    v_dram = nc.dram_tensor("V_scratch", [2, P, KC, 512], F32R).ap()

    with tile.TileContext(nc) as tc:
        with (
            tc.tile_pool(name="const", bufs=1) as cpool,
            tc.tile_pool(name="persist", bufs=1) as persist,
            tc.tile_pool(name="psumT4", bufs=3, space="PSUM") as psumT4,
            tc.tile_pool(name="psumMM", bufs=4, space="PSUM") as psumMM,
        ):
            ident = cpool.tile([P, P], F32, tag="ident")
            make_identity(nc, ident[:])

            if use_bias:
                # per-partition bias layouts: b[(eo p)] -> [p, eo]
                bk_sb = cpool.tile([P, EC], F32, tag="bk")
                nc.sync.dma_start(bk_sb[:], bk_d.rearrange("(o p) -> p o", p=P))
                bq_sb = cpool.tile([P, EC], F32, tag="bq")
                nc.sync.dma_start(bq_sb[:], bq_d.rearrange("(o p) -> p o", p=P))

                # ones-column trick operands for the V bias (free-dim bias):
                # onescol[p, m] = (p == 0); bvpad[0, :] = bv, others zero.
                # psum += onescol.T @ bvpad broadcasts bv to all partitions.
                onescol = cpool.tile([P, P], F32R, tag="onescol")
                bvpad = cpool.tile([P, D], F32R, tag="bvpad")

            # K^T [e, k] and Wq^T stay resident from phase A through phase B.
            KT = persist.tile([P, EC, LK], F32R, tag="KT")  # KT[p, eo, k]
            WqT = persist.tile([P, DC, D], F32R, tag="WqT")  # WqT[p, do, e]

            def dma_chunk_split(dst, src_row0, src_ap):
                """DMA a [P, D] natural chunk in two halves so transposes of
                the first half can start while the second half streams."""
                nc.sync.dma_start(
                    dst[:, 0 : D // 2],
                    src_ap[src_row0 : src_row0 + P, 0 : D // 2],
                )
                nc.sync.dma_start(
                    dst[:, D // 2 : D],
                    src_ap[src_row0 : src_row0 + P, D // 2 : D],
                )

            def transpose_batch(src, src_off, dst4, dst_c0, nblk):
                """PE-transpose `nblk` (<=4) contiguous [P, P] blocks of `src`
                starting at free-offset `src_off`, into dst4[:, dst_c0+j, :]
                via one shared PSUM bank and a single batched copy."""
                pst = psumT4.tile([P, 4, P], F32, tag="pT4")
                for j in range(nblk):
                    nc.tensor.transpose(
                        pst[:, j, :],
                        src[:, src_off + j * P : src_off + (j + 1) * P],
                        ident[:],
                    )
                nc.any.tensor_copy(
                    dst4[:, dst_c0 : dst_c0 + nblk, :], pst[:, :nblk, :]
                )

            # ------------- Phase A: K^T (SBUF) + V (DRAM) from one x pass ----
            with (
                tc.tile_pool(name="pa_nat", bufs=4) as natp,
                tc.tile_pool(name="pa_wT", bufs=1) as wTp,
                tc.tile_pool(name="pa_xT", bufs=1) as xTp,
                tc.tile_pool(name="pa_vstg", bufs=3) as vstgp,
            ):
                # PE warmup: ~3.5us of dummy fp32 matmuls on the identity so
                # the HAM clock-gate reaches 8/8 while the first DMAs stream.
                wps = psumMM.tile([P, 512], F32, tag="pMM")
                for w in range(8):
                    nc.tensor.matmul(
                        wps[:, 0:P],
                        ident[:],
                        ident[:],
                        start=(w == 0),
                        stop=(w == 7),
                    )

                if use_bias:
                    # f32r tiles can't be memset/DMA'd directly: stage in f32
                    # and let ACT copies do the f32->f32r rounding.
                    stage = natp.tile([P, D], F32, tag="nat")
                    nc.gpsimd.memset(stage[:], 0.0)
                    nc.gpsimd.memset(stage[0:1, 0:P], 1.0)
                    nc.scalar.copy(onescol[:], stage[:, 0:P])
                    bvstage = natp.tile([P, D], F32, tag="nat")
                    nc.gpsimd.memset(bvstage[:], 0.0)
                    nc.sync.dma_start(
                        bvstage[0:1, :], bv_d.rearrange("(a d) -> a d", a=1)
                    )
                    nc.scalar.copy(bvpad[:], bvstage[:])

                WkT = wTp.tile([P, DC, D], F32R, tag="WkT")
                WvT = wTp.tile([P, DC, D], F32R, tag="WvT")
                WkT4 = WkT[:].rearrange("p dc (g b) -> p (dc g) b", b=P)
                WvT4 = WvT[:].rearrange("p dc (g b) -> p (dc g) b", b=P)

                def transpose_w(wdram, wT4):
                    for eo in range(EC):
                        wchunk = natp.tile([P, D], F32, tag="nat")
                        dma_chunk_split(wchunk, eo * P, wdram)
                        for dc in range(DC):
                            transpose_batch(wchunk, dc * P, wT4, dc * EC + eo, 1)

                def transpose_x_block(kb):
                    xT = xTp.tile([P, DC, KB], F32R, tag="xT")
                    xT4 = xT[:].rearrange("p dc (g b) -> p (dc g) b", b=P)
                    for k4 in range(KB // P):
                        xchunk = natp.tile([P, D], F32, tag="nat")
                        dma_chunk_split(xchunk, kb * KB + k4 * P, x_d)
                        for dc in range(DC):
                            transpose_batch(
                                xchunk, dc * P, xT4, dc * (KB // P) + k4, 1
                            )
                    return xT

                def kt_block(kb, xT):
                    # KT[:, ec, kb] += sum_dc WkT[:, dc, ec].T @ xT[:, dc, :]
                    for ec in range(EC):
                        ps = psumMM.tile([P, 512], F32, tag="pMM")
                        for dc in range(DC):
                            nc.tensor.matmul(
                                ps[:, :KB],
                                WkT[:, dc, ec * P : (ec + 1) * P],
                                xT[:, dc, :],
                                start=(dc == 0),
                                stop=(dc == DC - 1),
                            )
                        if use_bias:
                            # copy + per-partition bias bk[e]
                            nc.scalar.activation(
                                KT[:, ec, kb * KB : (kb + 1) * KB],
                                ps[:, :KB],
                                Ident,
                                bias=bk_sb[:, ec : ec + 1],
                            )
                        else:
                            nc.any.tensor_copy(
                                KT[:, ec, kb * KB : (kb + 1) * KB], ps[:, :KB]
                            )

                def v_block(kb, xT):
                    # V[kb*4+k4] = sum_dc xT[:,dc,k4].T @ WvT[:,dc,:] (+ bv),
                    # staged out to DRAM for phase B
                    for k4 in range(KB // P):
                        ko = kb * (KB // P) + k4
                        for eh in range(2):
                            ps = psumMM.tile([P, 512], F32, tag="pMM")
                            if use_bias:
                                nc.tensor.matmul(
                                    ps[:],
                                    onescol[:],
                                    bvpad[:, eh * 512 : (eh + 1) * 512],
                                    start=True,
                                    stop=False,
                                )
                            for dc in range(DC):
                                nc.tensor.matmul(
                                    ps[:],
                                    xT[:, dc, k4 * P : (k4 + 1) * P],
                                    WvT[:, dc, eh * 512 : (eh + 1) * 512],
                                    start=(dc == 0 and not use_bias),
                                    stop=(dc == DC - 1),
                                )
                            vstg = vstgp.tile([P, 512], F32R, tag="vstg")
                            nc.any.tensor_copy(vstg[:], ps[:])
                            nc.sync.dma_start(v_dram[eh, :, ko, :], vstg[:])

                # Ordering: Wk streams first so KT matmuls can start after only
                # 4MB of W traffic; Wv streams while KT(kb0) computes.
                transpose_w(wk_d, WkT4)
                xT0 = transpose_x_block(0)
                kt_block(0, xT0)
                transpose_w(wv_d, WvT4)
                v_block(0, xT0)
                for kb in range(1, NKB):
                    xT = transpose_x_block(kb)
                    kt_block(kb, xT)
                    v_block(kb, xT)

                # build Wq^T at the tail of phase A: its DMA rides the DMA
                # slack here, and phase B then starts with operands ready
                WqT4 = WqT[:].rearrange("p dc (g b) -> p (dc g) b", b=P)
                transpose_w(wq_d, WqT4)

            # ------------- Phase B: per-q-block attention --------------------
            with (
                tc.tile_pool(name="pb_v", bufs=1) as vp,
                tc.tile_pool(name="pb_nat", bufs=2) as qnatp,
                tc.tile_pool(name="pb_small", bufs=1) as smallp,
                tc.tile_pool(name="pb_exp", bufs=1) as expp,
                tc.tile_pool(name="pb_attnT", bufs=1) as attnTp,
                tc.tile_pool(name="pb_out", bufs=1) as outp,
            ):
                # fetch V from the bounce buffer, e-half 0 first (the first
                # weighted matmul needs only half 0)
                V = vp.tile([P, KC, D], F32R, tag="V")  # V[p, ko, e]
                for eh in range(2):
                    nc.sync.dma_start(
                        V[:, :, eh * 512 : (eh + 1) * 512], v_dram[eh]
                    )

                # keep the PE warm across the phase boundary while the V/q
                # DMAs land
                wps = psumMM.tile([P, 512], F32, tag="pMM")
                for w in range(8):
                    nc.tensor.matmul(
                        wps[:, 0:P],
                        ident[:],
                        ident[:],
                        start=(w == 0),
                        stop=(w == 7),
                    )

                for qb in range(NQB):
                    qs = qb * P
                    # load q block, transpose to qT[d-part, q]
                    qchunk = qnatp.tile([P, D], F32, tag="qnat")
                    dma_chunk_split(qchunk, qs, q_d)
                    qT = smallp.tile([P, DC, P], F32R, tag="qT")
                    for g in range(DC // 4):
                        transpose_batch(qchunk, g * 4 * P, qT, g * 4, 4)

                    # Q natural [q, e] = sum_dc qT[:, dc].T @ WqT[:, dc, :]
                    Qn = smallp.tile([P, D], F32, tag="Qn")
                    for eh in range(2):
                        ps = psumMM.tile([P, 512], F32, tag="pMM")
                        for dc in range(DC):
                            nc.tensor.matmul(
                                ps[:],
                                qT[:, dc, :],
                                WqT[:, dc, eh * 512 : (eh + 1) * 512],
                                start=(dc == 0),
                                stop=(dc == DC - 1),
                            )
                        nc.any.tensor_copy(Qn[:, eh * 512 : (eh + 1) * 512], ps[:])

                    # transpose Q -> QT[e-part, q], adding bq (per-partition)
                    QT = smallp.tile([P, EC, P], F32R, tag="QT")
                    if use_bias:
                        for g in range(EC // 4):
                            pst = psumT4.tile([P, 4, P], F32, tag="pT4")
                            for j in range(4):
                                ec = g * 4 + j
                                nc.tensor.transpose(
                                    pst[:, j, :],
                                    Qn[:, ec * P : (ec + 1) * P],
                                    ident[:],
                                )
                            for j in range(4):
                                ec = g * 4 + j
                                nc.scalar.activation(
                                    QT[:, ec, :],
                                    pst[:, j, :],
                                    Ident,
                                    bias=bq_sb[:, ec : ec + 1],
                                )
                    else:
                        for g in range(EC // 4):
                            transpose_batch(Qn, g * 4 * P, QT, g * 4, 4)

                    # scores (psum) -> exp + row-sum, in chunks of 512
                    exp_sb = expp.tile([P, LK], F32, tag="exp")
                    sums4 = smallp.tile([P, 4], F32, tag="sums4")
                    for kq in range(LK // 512):
                        ps = psumMM.tile([P, 512], F32, tag="pMM")
                        for ec in range(EC):
                            nc.tensor.matmul(
                                ps[:],
                                QT[:, ec, :],
                                KT[:, ec, kq * 512 : (kq + 1) * 512],
                                start=(ec == 0),
                                stop=(ec == EC - 1),
                            )
                        nc.scalar.activation(
                            exp_sb[:, kq * 512 : (kq + 1) * 512],
                            ps[:],
                            Exp,
                            scale=SCALE,
                            accum_out=sums4[:, kq : kq + 1],
                        )

                    # transpose UNNORMALIZED exp -> attnT[k-part, q]; the
                    # 1/rowsum is applied on the weighted output instead,
                    # so the PE never waits on the softmax reduction.
                    attnT = attnTp.tile([P, KC, P], F32R, tag="attnT")
                    for g in range(KC // 4):
                        transpose_batch(exp_sb, g * 4 * P, attnT, g * 4, 4)

                    sumk = smallp.tile([P, 1], F32, tag="sumk")
                    nc.vector.reduce_sum(sumk[:], sums4[:], axis=AX)
                    rsum = smallp.tile([P, 1], F32, tag="rsum")
                    nc.vector.reciprocal(rsum[:], sumk[:])
                    # normalize in place (after the transposes read it),
                    # write attention out
                    nc.vector.tensor_scalar_mul(exp_sb[:], exp_sb[:], rsum[:])
                    nc.sync.dma_start(attn_d[qs : qs + P, :], exp_sb[:])

                    # weighted [q, e] = (sum_kc attnT[:, kc].T @ V) * rsum
                    wout = outp.tile([P, D], F32, tag="wout")
                    for eh in range(2):
                        ps = psumMM.tile([P, 512], F32, tag="pMM")
                        for kc in range(KC):
                            nc.tensor.matmul(
                                ps[:],
                                attnT[:, kc, :],
                                V[:, kc, eh * 512 : (eh + 1) * 512],
                                start=(kc == 0),
                                stop=(kc == KC - 1),
                            )
                        nc.scalar.activation(
                            wout[:, eh * 512 : (eh + 1) * 512],
                            ps[:],
                            Copy,
                            scale=rsum[:],
                        )
                    nc.sync.dma_start(out_d[qs : qs + P, :], wout[:])

    nc.compile()
    return nc


def _get_nc(use_bias=True):
    key = ("nc", use_bias)
    if key not in _CACHE:
        _CACHE[key] = build_nc(use_bias=use_bias)
    return _CACHE[key]


def kernel(**inputs):
    from concourse.bass_utils import run_bass_kernel_spmd

    use_bias = any(
        np.any(np.asarray(inputs[k])) for k in ("bq", "bk", "bv")
    )
    nc = _get_nc(use_bias=use_bias)
    in_maps = []
    for b in range(B):
        in_maps.append(
            {
                "x": np.ascontiguousarray(inputs["x"][b], dtype=np.float32),
                "q": np.ascontiguousarray(inputs["q"][b], dtype=np.float32),
                "Wq": np.asarray(inputs["Wq"], dtype=np.float32),
                "bq": np.asarray(inputs["bq"], dtype=np.float32),
                "Wk": np.asarray(inputs["Wk"], dtype=np.float32),
                "bk": np.asarray(inputs["bk"], dtype=np.float32),
                "Wv": np.asarray(inputs["Wv"], dtype=np.float32),
                "bv": np.asarray(inputs["bv"], dtype=np.float32),
            }
        )
    res = run_bass_kernel_spmd(nc, in_maps, core_ids=list(range(B)))
    weighted = np.stack([res.results[b]["weighted"] for b in range(B)])
    attention = np.stack([res.results[b]["attention"] for b in range(B)])
    return weighted, attention
